# revision 1
# baseline (speedup 1.0000x reference)
"""Trainium2 Bass kernel for the pixel-RNN (tanh RNN, T=784, H=512, B=256).

Strategy: data-parallel over batch (32 samples per core, 8 cores).
Per core, per time step:
  - PSUM split into two j-halves [32, 256] (separate banks).
  - x-term: K=2 matmul  [x_t; 1]^T @ [w_ih; b_ih+b_hh]  (start=True).
  - recurrence: 4 k-chunk matmuls per half, lhsT = hT chunk [128,32]
    (stationary, cheap), rhs = W_hh^T chunk [128,256] (moving, fp32r at
    1 cycle/col for N>=256).
  - tanh on ScalarE (PSUM -> SBUF h [32,512]).
  - h -> hT via 4 PE transposes (PSUM) + VectorE copies (SBUF).
Final linear head (10 classes) on device; log-softmax / loss / argmax
on host (tiny [256,10] reduction).

A (self-loading) fp32r matmul can carry at most ONE sync wait in codegen,
and each dma_start lands on its own DMA queue (own semaphore). So after the
constant DMAs, one tiny "gate" matmul per DMA absorbs that queue's semaphore
into the PE's observed clock; every later matmul then needs at most one wait.
"""

import sys

if "/opt/trn_rl_repo" not in sys.path:
    sys.path.insert(0, "/opt/trn_rl_repo")

import numpy as np

B, T, H, NCLS = 256, 784, 512, 10
NCORES = 8
BC = B // NCORES   # 32 samples per core
KC = H // 128      # 4 contraction chunks
JH = H // 2        # 256, j-half width

_BUILD_CACHE = {}


def _build(t_steps=T, split_waits=True):
    """Build the Bass module (single program, run SPMD on 8 cores)."""
    import concourse.bass as bass
    import concourse.mybir as mybir
    from concourse import tile
    from concourse.tile_rust import add_dep_helper

    f32 = mybir.dt.float32
    f32r = mybir.dt.float32r
    Tanh = mybir.ActivationFunctionType.Tanh

    nc = bass.Bass(
        "TRN2",
        target_bir_lowering=False,
        debug=False,
        enable_asserts=False,
        num_devices=NCORES,
    )

    d_xT = nc.dram_tensor("xT", (2, t_steps * BC), f32r, kind="ExternalInput").ap()
    d_wihb = nc.dram_tensor("wihb", (2, H), f32r, kind="ExternalInput").ap()
    d_WT = nc.dram_tensor("WT", (128, KC * H), f32r, kind="ExternalInput").ap()
    d_lWT = nc.dram_tensor("lWT", (128, KC * NCLS), f32r, kind="ExternalInput").ap()
    d_id = nc.dram_tensor("ident", (32, 32), f32, kind="ExternalInput").ap()
    d_out = nc.dram_tensor("logitsT", (NCLS, BC), f32, kind="ExternalOutput").ap()

    with tile.TileContext(nc) as tc:
        with (
            tc.tile_pool(name="const", bufs=1) as cpool,
            tc.tile_pool(name="ps", bufs=1, space="PSUM") as ppool,
        ):
            xT_sb = cpool.tile([2, t_steps * BC], f32r, tag="xT")
            wihb_sb = cpool.tile([2, H], f32r, tag="wihb")
            WT_sb = cpool.tile([128, KC * H], f32r, tag="WT")
            lWT_sb = cpool.tile([128, KC * NCLS], f32r, tag="lWT")
            id_sb = cpool.tile([32, 32], f32, tag="ident")
            out_sb = cpool.tile([NCLS, BC], f32, tag="out")

            # ping-pong working set: allocated once -> no tile-slot releases,
            # so every hot-path instruction needs at most one sync wait.
            hh = [cpool.tile([BC, H], f32, tag=f"h{p}", name=f"h{p}")
                  for p in range(2)]
            hT = [cpool.tile([128, KC * BC], f32r, tag=f"hT{p}", name=f"hT{p}")
                  for p in range(2)]
            # PSUM: per parity, 2 accumulators [32,256] + 2 transpose banks
            ph = [[ppool.tile([BC, JH], f32, tag=f"ph{p}{i}", name=f"ph{p}{i}")
                   for i in range(2)] for p in range(2)]
            pt = [[ppool.tile([128, 2 * BC], f32, tag=f"pt{p}{i}",
                              name=f"pt{p}{i}") for i in range(2)]
                  for p in range(2)]

            nc.sync.dma_start(out=xT_sb[:, :], in_=d_xT)
            nc.sync.dma_start(out=wihb_sb[:, :], in_=d_wihb)
            for kc in range(KC):
                nc.sync.dma_start(
                    out=WT_sb[:, kc * H:(kc + 1) * H],
                    in_=d_WT[:, kc * H:(kc + 1) * H],
                )
            nc.sync.dma_start(out=lWT_sb[:, :], in_=d_lWT)
            nc.sync.dma_start(out=id_sb[:, :], in_=d_id)

            # gate matmuls: one per DMA, each absorbing one queue semaphore
            # into the PE's observed clock (results discarded)
            gates = [
                (xT_sb[0:2, 0:BC], xT_sb[0:2, 0:JH]),
                (wihb_sb[0:2, 0:BC], wihb_sb[0:2, 0:JH]),
            ]
            for kc in range(KC):
                gates.append(
                    (WT_sb[:, kc * H:kc * H + BC], WT_sb[:, kc * H:kc * H + JH])
                )
            gates.append((lWT_sb[:, 0:32], lWT_sb[:, 0:KC * NCLS]))
            for glhs, grhs in gates:
                nc.tensor.matmul(ph[0][0][:, 0:grhs.shape[-1]], glhs, grhs,
                                 start=True, stop=True)
            nc.tensor.matmul(
                ph[0][0][0:32, 0:32], id_sb[:, 0:32], id_sb[:, :],
                start=True, stop=True,
            )

            for t in range(t_steps):
                p, q = t % 2, 1 - (t % 2)
                xlhs = xT_sb[0:2, t * BC:(t + 1) * BC]
                first = t == 0
                # emit half0's x-term + full contraction before touching
                # half1: psum half0 completes one matmul-slot earlier, so the
                # tanh0 -> transpose -> copy0 chain (which feeds the next
                # step's first matmuls) starts earlier.
                for half in range(2):
                    nc.tensor.matmul(
                        ph[p][half][:, :],
                        xlhs,
                        wihb_sb[0:2, half * JH:(half + 1) * JH],
                        start=True,
                        stop=first,
                    )
                    if not first:
                        for kc in range(KC):
                            nc.tensor.matmul(
                                ph[p][half][:, :],
                                hT[q][:, kc * BC:(kc + 1) * BC],
                                WT_sb[
                                    :, kc * H + half * JH: kc * H + (half + 1) * JH
                                ],
                                start=False,
                                stop=(kc == KC - 1),
                            )

                for half in range(2):
                    nc.scalar.activation(
                        hh[p][:, half * JH:(half + 1) * JH], ph[p][half][:, :],
                        Tanh,
                    )

                for i in range(2):
                    for j in range(2):
                        kc = 2 * i + j
                        nc.tensor.transpose(
                            pt[p][i][:, j * BC:(j + 1) * BC],
                            hh[p][0:BC, kc * 128:(kc + 1) * 128],
                            id_sb[:, :],
                        )
                    nc.vector.tensor_copy(
                        hT[p][:, i * 2 * BC:(i + 1) * 2 * BC], pt[p][i][:, :]
                    )

            # final linear head: logitsT[c, b] = sum_j lin_W[c, j] h[b, j]
            pl = (t_steps - 1) % 2
            pL = ph[1 - pl][0]
            for kc in range(KC):
                nc.tensor.matmul(
                    pL[0:NCLS, 0:BC],
                    lWT_sb[:, kc * NCLS:(kc + 1) * NCLS],
                    hT[pl][:, kc * BC:(kc + 1) * BC],
                    start=(kc == 0),
                    stop=(kc == KC - 1),
                )
            nc.vector.tensor_copy(out_sb[:, :], pL[0:NCLS, 0:BC])
            nc.sync.dma_start(out=d_out, in_=out_sb[:, :])

    if split_waits:
        _split_multi_waits(nc, mybir)
    return nc


def _split_multi_waits(nc, mybir):
    """Walrus can pack only one sync wait into a HW instruction. Move any
    extra waits onto same-engine NoOps inserted right before (the engine's
    sequencer executes them in order, so semantics are unchanged)."""
    nid = 0
    for b in nc.m.functions[0].blocks:
        out = []
        changed = False
        for ins in b.instructions:
            si = getattr(ins, "sync_info", None)
            ws = list(getattr(si, "on_wait", []) or []) if si else []
            if len(ws) > 1:
                for w in ws[:-1]:
                    nid += 1
                    out.append(mybir.InstNoOp(
                        name=f"I-wsplit-{nid}",
                        engine=ins.engine,
                        sync_info=mybir.SyncInfo(on_wait=[w], on_update=[]),
                    ))
                ins.sync_info = mybir.SyncInfo(
                    on_wait=[ws[-1]], on_update=list(si.on_update or [])
                )
                changed = True
            out.append(ins)
        if changed:
            b.instructions = out


def _pack_inputs(inputs, order, W_ih, b_ih, W_hh, b_hh, lin_W, t_steps=T):
    """Host-side shard packing: returns in_maps list (one dict per core)."""
    x = np.asarray(inputs, np.float32)[:, np.asarray(order, np.int64)]
    x = np.ascontiguousarray(x[:, :t_steps])
    wihb = np.stack(
        [np.asarray(W_ih, np.float32)[:, 0],
         np.asarray(b_ih, np.float32) + np.asarray(b_hh, np.float32)]
    )  # [2, H]
    WT = np.ascontiguousarray(
        np.asarray(W_hh, np.float32).T.reshape(KC, 128, H)
        .transpose(1, 0, 2).reshape(128, KC * H)
    )
    lWT = np.ascontiguousarray(
        np.asarray(lin_W, np.float32).T.reshape(KC, 128, NCLS)
        .transpose(1, 0, 2).reshape(128, KC * NCLS)
    )
    ident = np.eye(32, dtype=np.float32)


    in_maps = []
    for c in range(NCORES):
        xc = x[c * BC:(c + 1) * BC]  # [BC, t]
        xT = np.ones((2, t_steps * BC), np.float32)
        xT[0] = xc.T.reshape(-1)
        in_maps.append(
            {"xT": xT, "wihb": wihb, "WT": WT, "lWT": lWT, "ident": ident}
        )
    return in_maps


def _run(inputs, y, order, W_ih, b_ih, W_hh, b_hh, lin_W, lin_b, trace=False):
    from concourse import bass_utils

    key = T
    if key not in _BUILD_CACHE:
        _BUILD_CACHE[key] = _build(T)
    nc = _BUILD_CACHE[key]

    in_maps = _pack_inputs(inputs, order, W_ih, b_ih, W_hh, b_hh, lin_W, T)
    res = bass_utils.run_bass_kernel_spmd(
        nc, in_maps, core_ids=list(range(NCORES)), trace=trace
    )

    logits = np.empty((B, NCLS), np.float32)
    for c in range(NCORES):
        logits[c * BC:(c + 1) * BC] = res.results[c]["logitsT"].T
    logits = logits + np.asarray(lin_b, np.float32)[None, :]

    yv = np.asarray(y).astype(np.int64)
    m = logits.max(axis=1, keepdims=True)
    logp = logits - (np.log(np.exp(logits - m).sum(axis=1, keepdims=True)) + m)
    loss = np.float32(-logp[np.arange(B), yv].mean())
    correct = np.int32((logits.argmax(axis=1) == yv).sum())
    return (loss, correct), res


def kernel(inputs, y, order, W_ih, b_ih, W_hh, b_hh, lin_W, lin_b):
    out, _ = _run(inputs, y, order, W_ih, b_ih, W_hh, b_hh, lin_W, lin_b)
    return out

